# revision 7
# baseline (speedup 1.0000x reference)
"""BasicTransformerBlock on 8 TRN2 NeuronCores.

Sharding: data-parallel, core = (batch b in 0..3) x (sequence half h in 0..1).
Each core receives its batch element's full sequence rotated so its local 512
rows come first (softmax over keys is permutation invariant), computes K/V of
attn1 for all 1024 tokens (duplicated across the pair, ~10% extra FLOPs, zero
collectives), and everything else for its 512 local tokens only.

On-chip layout: feature-major activations [features on partitions, tokens on
free axis] so every projection consumes natural-layout weights as the matmul
stationary operand. Matmuls run in fp16 (weights pre-cast on host); the
residual stream, LN math and PSUM accumulation stay fp32. LayerNorm partition
reductions and per-token broadcasts use float32r ones-matmuls. Attention
softmax denominators come free from a ones-column appended to V.
"""

import sys
import types

sys.path.insert(0, "/opt/trn_rl_repo")

# concourse fetches the NTFF profile hook from antenv.axon_hooks, which the
# agent image's antenv stub lacks. Register a shim so trace=True works.
if "antenv.axon_hooks" not in sys.modules:
    _hooks = types.ModuleType("antenv.axon_hooks")
    _HOOK = [None]

    def _get_hook():
        if _HOOK[0] is None:
            try:
                from trn_agent_boot.trn_boot import _ntff_profile_via_ctypes

                _HOOK[0] = _ntff_profile_via_ctypes("/opt/axon/libaxon_pjrt.so")
            except Exception:
                _HOOK[0] = None
        return _HOOK[0]

    _hooks.get_axon_ntff_profile_hook = _get_hook
    _hooks.set_axon_ntff_profile_hook = lambda h: _HOOK.__setitem__(0, h)
    sys.modules["antenv.axon_hooks"] = _hooks
    try:
        import antenv

        antenv.axon_hooks = _hooks
    except ImportError:
        pass

import numpy as np

import concourse.bass as bass
import concourse.mybir as mybir
import concourse.tile as tile
from concourse import bacc, bass_utils

dt = mybir.dt
F32, F16, F32R = dt.float32, dt.float16, dt.float32r
AF = mybir.ActivationFunctionType

DIM, HEADS, DHEAD, CTX_DIM, DFF = 1280, 20, 64, 768, 5120
BATCH, NTOK, MCTX = 4, 1024, 77
EPS = 1e-5
SCALE = DHEAD ** -0.5
N_CORES = 8
T = 512         # local tokens per core
TKV = 1024      # attn1 key/value tokens per core
KC = DIM // 128           # 10
KCX = CTX_DIM // 128      # 6
JFF = DFF // 128          # 40 (chunks of the gated hidden)
P = 128

last_exec_time_ns = None


def _emit(tc, d, trivial_aff, trivial_bias):
    nc = tc.nc
    pools = {}

    def pool(name, bufs, space="SBUF", side="left"):
        p = tc.alloc_tile_pool(name=name, bufs=bufs, space=space, side=side)
        pools[name] = p
        return p

    def close(*names):
        for n in names:
            pools.pop(n).release()

    # Pools are two LIFO stacks (left/right) per memory space; lifetimes below
    # are arranged so every release pops the top of its stack.
    const = pool("const", 1)
    ones_col = const.tile([P, 1], F16, name="ones_col")
    nc.vector.memset(ones_col[:], 1.0)
    ones_row = const.tile([1, P], F16, name="ones_row")
    nc.vector.memset(ones_row[:], 1.0)
    if not trivial_aff:
        aff = const.tile([P, 60], F32, name="aff")
        nc.sync.dma_start(aff[:], d["aff"])
    if not trivial_bias:
        biases = const.tile([P, 110], F32, name="biases")
        nc.sync.dma_start(biases[:], d["biases"])

    tmp = pool("tmp", 1)

    # ---------------- helpers ----------------

    def layernorm(x_tiles, Ttok, ln_idx, out_tiles, ln_psum):
        """x_tiles: KC SBUF tiles [128, Ttok] f32 -> out_tiles [128, Ttok] fp16."""
        for t in range(Ttok // 512):
            sl = slice(t * 512, (t + 1) * 512)
            sums_ps = ln_psum.tile([1, 512], F32, name=f"lns{ln_idx}_{t}", tag="lnstat", bufs=2)
            sq_ps = ln_psum.tile([1, 512], F32, name=f"lnq{ln_idx}_{t}", tag="lnstat", bufs=2)
            for c in range(KC):
                xh = tmp.tile([P, 512], F16, name=f"xh{ln_idx}_{t}_{c}", tag="xh", bufs=3)
                nc.any.tensor_copy(out=xh[:], in_=x_tiles[c][:, sl])
                xsq = tmp.tile([P, 512], F16, name=f"xsq{ln_idx}_{t}_{c}", tag="xsq", bufs=3)
                nc.vector.tensor_mul(xsq[:], x_tiles[c][:, sl], x_tiles[c][:, sl])
                nc.tensor.matmul(sums_ps[:], ones_col[:], xh[:],
                                 start=(c == 0), stop=(c == KC - 1))
                nc.tensor.matmul(sq_ps[:], ones_col[:], xsq[:],
                                 start=(c == 0), stop=(c == KC - 1))
            ssum = tmp.tile([1, 512], F16, name=f"ssum{ln_idx}_{t}", tag="ssum", bufs=2)
            nc.scalar.copy(ssum[:], sums_ps[:])
            ssq = tmp.tile([1, 512], F16, name=f"ssq{ln_idx}_{t}", tag="ssq", bufs=2)
            nc.scalar.copy(ssq[:], sq_ps[:])
            bs_ps = ln_psum.tile([P, 512], F32, name=f"bs{ln_idx}_{t}", tag="lnbc", bufs=2)
            nc.tensor.matmul(bs_ps[:], ones_row[:], ssum[:], start=True, stop=True)
            bq_ps = ln_psum.tile([P, 512], F32, name=f"bq{ln_idx}_{t}", tag="lnbc", bufs=2)
            nc.tensor.matmul(bq_ps[:], ones_row[:], ssq[:], start=True, stop=True)
            mu = tmp.tile([P, 512], F32, name=f"mu{ln_idx}_{t}", tag="mu", bufs=2)
            nc.vector.tensor_scalar_mul(mu[:], bs_ps[:], 1.0 / DIM)
            musq = tmp.tile([P, 512], F32, name=f"musq{ln_idx}_{t}", tag="musq", bufs=2)
            nc.vector.tensor_mul(musq[:], mu[:], mu[:])
            # musq - EPS, so var = ex2 - musq + EPS below
            nc.vector.tensor_scalar_sub(musq[:], musq[:], EPS)
            var = tmp.tile([P, 512], F32, name=f"var{ln_idx}_{t}", tag="var", bufs=2)
            nc.vector.scalar_tensor_tensor(var[:], bq_ps[:], 1.0 / DIM, musq[:],
                                           mybir.AluOpType.mult, mybir.AluOpType.subtract)
            std = tmp.tile([P, 512], F32, name=f"std{ln_idx}_{t}", tag="std", bufs=2)
            nc.scalar.sqrt(std[:], var[:])
            rstd = tmp.tile([P, 512], F32, name=f"rstd{ln_idx}_{t}", tag="rstd", bufs=2)
            nc.vector.reciprocal(rstd[:], std[:])
            for c in range(KC):
                xm = tmp.tile([P, 512], F32, name=f"xm{ln_idx}_{t}_{c}", tag="xm", bufs=3)
                nc.vector.tensor_sub(xm[:], x_tiles[c][:, sl], mu[:])
                if trivial_aff:
                    nc.vector.tensor_mul(out_tiles[c][:, sl], xm[:], rstd[:])
                else:
                    xn = tmp.tile([P, 512], F32, name=f"xn{ln_idx}_{t}_{c}", tag="xn", bufs=3)
                    nc.vector.tensor_mul(xn[:], xm[:], rstd[:])
                    g_ap = aff[:, ln_idx * 20 + c: ln_idx * 20 + c + 1]
                    be_ap = aff[:, ln_idx * 20 + 10 + c: ln_idx * 20 + 10 + c + 1]
                    xg = tmp.tile([P, 512], F32, name=f"xg{ln_idx}_{t}_{c}", tag="xg", bufs=3)
                    nc.vector.tensor_scalar_mul(xg[:], xn[:], g_ap)
                    nc.scalar.activation(out_tiles[c][:, sl], xg[:], AF.Copy, bias=be_ap)

    def attention(c, h, Kt_c, Qt_c, Vt, n_kc, kv_par, sc_psum, ov_psum, epool):
        """One head. Kt_c/Qt_c fp16 feature-major with heads 2c/2c+1 on
        partition halves; Vt: n_kc token-major [*,20,65] fp16 tiles."""
        head = 2 * c + h
        hp = slice(64 * h, 64 * h + 64)
        exps = []
        for k8 in range(n_kc):
            sps = sc_psum.tile([kv_par, 512], F32, name=f"sps{head}_{k8}", tag="sc", bufs=4)
            nc.tensor.matmul(sps[:], Kt_c[hp, k8 * kv_par:(k8 + 1) * kv_par], Qt_c[hp, :],
                             start=True, stop=True, tile_position=(64 * h, 0))
            e = epool.tile([kv_par, 512], F16, name=f"exp{head}_{k8}", tag="exp")
            nc.scalar.activation(e[:], sps[:], AF.Exp, scale=SCALE)
            exps.append(e)
        ops_ = ov_psum.tile([DHEAD + 1, 512], F32, name=f"ov{head}", tag="ov", bufs=2)
        for k8 in range(n_kc):
            nc.tensor.matmul(ops_[:], Vt[k8][:kv_par, head, :], exps[k8][:],
                             start=(k8 == 0), stop=(k8 == n_kc - 1))
        return ops_

    def attn_finish(head, ops_, ov_psum, out_ap):
        rec32 = tmp.tile([1, 512], F32, name=f"rec32_{head}", tag="rec32", bufs=3)
        nc.vector.reciprocal(rec32[:], ops_[DHEAD:DHEAD + 1, :])
        rec = tmp.tile([1, 512], F16, name=f"rec{head}", tag="rec", bufs=3)
        nc.scalar.copy(rec[:], rec32[:])
        bps = ov_psum.tile([DHEAD, 512], F32, name=f"bps{head}", tag="obc", bufs=2)
        nc.tensor.matmul(bps[:], ones_row[:, :DHEAD], rec[:],
                         start=True, stop=True)
        bsb = tmp.tile([DHEAD, 512], F32, name=f"bsb{head}", tag="bsb", bufs=3)
        nc.scalar.copy(bsb[:], bps[:])
        nc.vector.tensor_mul(out_ap, ops_[:DHEAD, :], bsb[:])

    def project(w_d, n_kc, rhs_fn, n_mc, consume, wpool, wtag, psum_p, wbufs=3):
        """out[mc] = sum_kc w[mc][:, kc].T @ rhs(kc); consume(mc, psum)."""
        for mc in range(n_mc):
            wt = wpool.tile([P, n_kc, P], F16, name=f"{wtag}_{mc}", tag=wtag, bufs=wbufs)
            nc.sync.dma_start(wt[:], w_d[mc])
            ps = psum_p.tile([P, 512], F32, name=f"ps_{wtag}_{mc}", tag="proj", bufs=4)
            for kc in range(n_kc):
                nc.tensor.matmul(ps[:], wt[:, kc], rhs_fn(kc),
                                 start=(kc == 0), stop=(kc == n_kc - 1))
            consume(mc, ps)

    def bias_ap(col):
        return biases[:, col:col + 1]

    # ---------------- phase 1: load x, LN1 ----------------

    ln1p = pool("ln1p", 1)
    ln1t = [ln1p.tile([P, TKV], F16, name=f"ln1_{c}", tag="ln1", bufs=KC) for c in range(KC)]

    ln_psum = pool("ln_psum", 1, space="PSUM")
    xpool = pool("xpool", 1, side="right")
    x_sb = []
    for c in range(KC):
        xc = xpool.tile([P, TKV], F32, name=f"x_{c}", tag="x", bufs=KC)
        nc.sync.dma_start(xc[:], d["xt"][c * P:(c + 1) * P, :])
        x_sb.append(xc)
    layernorm(x_sb, TKV, 0, ln1t, ln_psum)
    close("xpool", "ln_psum")

    # ---------------- phase 2: Q, K, V projections ----------------

    proj_psum = pool("proj_psum", 1, space="PSUM")
    wpool = pool("wpool1", 1)
    qkv = pool("qkv", 1, side="right")

    Qt = [qkv.tile([P, T], F16, name=f"qt_{mc}", tag="qt", bufs=KC) for mc in range(KC)]
    Kt = [qkv.tile([P, TKV], F16, name=f"kt_{mc}", tag="kt", bufs=KC) for mc in range(KC)]
    Vt = [qkv.tile([P, HEADS, DHEAD + 1], F16, name=f"vt_{t8}", tag="vt", bufs=8)
          for t8 in range(8)]

    def q_consume(mc, ps):
        nc.any.tensor_copy(out=Qt[mc][:], in_=ps[:])

    project(d["wq1"], KC, lambda kc: ln1t[kc][:, 0:T], KC, q_consume, wpool, "wq1", proj_psum)

    for thalf in range(2):
        sl = slice(thalf * 512, (thalf + 1) * 512)

        def k_consume(mc, ps, sl=sl):
            nc.any.tensor_copy(out=Kt[mc][:, sl], in_=ps[:])

        project(d["wk1"], KC, lambda kc, sl=sl: ln1t[kc][:, sl], KC, k_consume, wpool, "wk1",
                proj_psum)

    # V token-major with a ones column per head (softmax denominator trick)
    wv_sb = []
    for kc in range(KC):
        wv = wpool.tile([P, DIM], F16, name=f"wv1_{kc}", tag="wv1", bufs=KC)
        nc.sync.dma_start(wv[:], d["wv1"][kc])
        wv_sb.append(wv)
    for t8 in range(8):
        nc.vector.memset(Vt[t8][:], 1.0)
        for n0, nsz in ((0, 512), (512, 512), (1024, 256)):
            ps = proj_psum.tile([P, 512], F32, name=f"psv_{t8}_{n0}", tag="proj", bufs=4)
            for kc in range(KC):
                nc.tensor.matmul(ps[:, :nsz], ln1t[kc][:, t8 * P:(t8 + 1) * P],
                                 wv_sb[kc][:, n0:n0 + nsz],
                                 start=(kc == 0), stop=(kc == KC - 1))
            nc.any.tensor_copy(
                out=Vt[t8][:, n0 // DHEAD:(n0 + nsz) // DHEAD, 0:DHEAD],
                in_=ps[:, :nsz].rearrange("p (h e) -> p h e", e=DHEAD))

    close("wpool1", "ln1p", "proj_psum")

    # ---------------- phase 3: attn1 ----------------

    otp = pool("otp", 1)
    Ot = [otp.tile([P, T], F16, name=f"ot_{c}", tag="ot", bufs=KC) for c in range(KC)]
    sc_psum = pool("sc_psum", 1, space="PSUM")
    ov_psum = pool("ov_psum", 1, space="PSUM")
    epool = pool("epool", 24, side="right")

    for c in range(KC):
        ops0 = attention(c, 0, Kt[c], Qt[c], Vt, 8, P, sc_psum, ov_psum, epool)
        ops1 = attention(c, 1, Kt[c], Qt[c], Vt, 8, P, sc_psum, ov_psum, epool)
        attn_finish(2 * c, ops0, ov_psum, Ot[c][0:DHEAD, :])
        attn_finish(2 * c + 1, ops1, ov_psum, Ot[c][DHEAD:2 * DHEAD, :])

    close("epool", "qkv", "ov_psum", "sc_psum")

    # ---------------- phase 4: out-proj 1 + residual ----------------

    resp = pool("resp", 1)
    wpool = pool("wpool2", 1)
    proj_psum = pool("proj_psum2", 1, space="PSUM")
    x1p = pool("x1p", 1, side="right")
    x1 = [x1p.tile([P, T], F32, name=f"x1_{mc}", tag="x1", bufs=KC) for mc in range(KC)]
    resid = []
    for c in range(KC):
        rc = resp.tile([P, T], F32, name=f"res_{c}", tag="res", bufs=KC)
        nc.sync.dma_start(rc[:], d["xt"][c * P:(c + 1) * P, 0:T])
        resid.append(rc)

    def o1_consume(mc, ps):
        if trivial_bias:
            nc.vector.tensor_add(x1[mc][:], ps[:], resid[mc][:])
        else:
            nc.vector.scalar_tensor_tensor(x1[mc][:], ps[:], bias_ap(mc), resid[mc][:],
                                           mybir.AluOpType.add, mybir.AluOpType.add)

    project(d["wo1"], KC, lambda kc: Ot[kc][:], KC, o1_consume, wpool, "wo1", proj_psum)
    close("wpool2", "resp", "otp", "proj_psum2")

    # ---------------- phase 5: LN2 + attn2 projections ----------------

    ln_psum = pool("ln_psum2", 1, space="PSUM")
    ln2p = pool("ln2p", 1)
    ln2t = [ln2p.tile([P, T], F16, name=f"ln2_{c}", tag="ln2", bufs=KC) for c in range(KC)]
    layernorm(x1, T, 1, ln2t, ln_psum)
    close("ln_psum2")

    proj_psum = pool("proj_psum2b", 1, space="PSUM")
    qkv2 = pool("qkv2", 1, side="right")
    wpool = pool("wpool2b", 1)
    ctx_sb = []
    for c in range(KCX):
        cc = qkv2.tile([P, MCTX], F32, name=f"ctx_{c}", tag="ctx", bufs=KCX)
        nc.sync.dma_start(cc[:], d["ctxt"][c * P:(c + 1) * P, :])
        ch = qkv2.tile([P, MCTX], F16, name=f"ctxh_{c}", tag="ctxh", bufs=KCX)
        nc.any.tensor_copy(out=ch[:], in_=cc[:])
        ctx_sb.append(ch)

    Q2t = [qkv2.tile([P, T], F16, name=f"q2t_{mc}", tag="q2t", bufs=KC) for mc in range(KC)]
    K2t = [qkv2.tile([P, MCTX], F16, name=f"k2t_{mc}", tag="k2t", bufs=KC) for mc in range(KC)]
    V2t = [qkv2.tile([P, HEADS, DHEAD + 1], F16, name="v2t", tag="v2t", bufs=1)]

    def q2_consume(mc, ps):
        nc.any.tensor_copy(out=Q2t[mc][:], in_=ps[:])

    project(d["wq2"], KC, lambda kc: ln2t[kc][:], KC, q2_consume, wpool, "wq2", proj_psum)

    for mc in range(KC):
        wt = wpool.tile([P, KCX, P], F16, name=f"wk2_{mc}", tag="wk2", bufs=3)
        nc.sync.dma_start(wt[:], d["wk2"][mc])
        ps = proj_psum.tile([P, MCTX], F32, name=f"psk2_{mc}", tag="projx", bufs=2)
        for kc in range(KCX):
            nc.tensor.matmul(ps[:], wt[:, kc], ctx_sb[kc][:], start=(kc == 0),
                             stop=(kc == KCX - 1))
        nc.any.tensor_copy(out=K2t[mc][:], in_=ps[:])

    wv2_sb = []
    for kc in range(KCX):
        wv = wpool.tile([P, DIM], F16, name=f"wv2_{kc}", tag="wv2", bufs=KCX)
        nc.sync.dma_start(wv[:], d["wv2"][kc])
        wv2_sb.append(wv)
    nc.vector.memset(V2t[0][:], 1.0)
    for n0, nsz in ((0, 512), (512, 512), (1024, 256)):
        ps = proj_psum.tile([MCTX, 512], F32, name=f"psv2_{n0}", tag="proj", bufs=4)
        for kc in range(KCX):
            nc.tensor.matmul(ps[:, :nsz], ctx_sb[kc][:], wv2_sb[kc][:, n0:n0 + nsz],
                             start=(kc == 0), stop=(kc == KCX - 1))
        nc.any.tensor_copy(
            out=V2t[0][:MCTX, n0 // DHEAD:(n0 + nsz) // DHEAD, 0:DHEAD],
            in_=ps[:, :nsz].rearrange("p (h e) -> p h e", e=DHEAD))

    close("wpool2b", "ln2p", "proj_psum2b")

    # ---------------- phase 6: attn2 ----------------

    o2p = pool("o2p", 1)
    O2t = [o2p.tile([P, T], F16, name=f"o2t_{c}", tag="o2t", bufs=KC) for c in range(KC)]
    sc_psum = pool("sc_psum2", 1, space="PSUM")
    ov_psum = pool("ov_psum2", 1, space="PSUM")
    epool = pool("epool2", 6, side="right")

    for c in range(KC):
        ops0 = attention(c, 0, K2t[c], Q2t[c], V2t, 1, MCTX, sc_psum, ov_psum, epool)
        ops1 = attention(c, 1, K2t[c], Q2t[c], V2t, 1, MCTX, sc_psum, ov_psum, epool)
        attn_finish(2 * c + 40, ops0, ov_psum, O2t[c][0:DHEAD, :])
        attn_finish(2 * c + 41, ops1, ov_psum, O2t[c][DHEAD:2 * DHEAD, :])

    close("epool2", "qkv2", "ov_psum2", "sc_psum2")

    # ---------------- phase 7: out-proj 2 + residual ----------------

    x2p = pool("x2p", 1)
    wpool = pool("wpool3", 1)
    proj_psum = pool("proj_psum3", 1, space="PSUM")
    x2 = [x2p.tile([P, T], F32, name=f"x2_{mc}", tag="x2", bufs=KC) for mc in range(KC)]

    def o2_consume(mc, ps):
        if trivial_bias:
            nc.vector.tensor_add(x2[mc][:], ps[:], x1[mc][:])
        else:
            nc.vector.scalar_tensor_tensor(x2[mc][:], ps[:], bias_ap(10 + mc), x1[mc][:],
                                           mybir.AluOpType.add, mybir.AluOpType.add)

    project(d["wo2"], KC, lambda kc: O2t[kc][:], KC, o2_consume, wpool, "wo2", proj_psum)
    close("wpool3", "x1p", "proj_psum3")

    # ---------------- phase 8: LN3 + GEGLU FF ----------------

    hhp = pool("hhp", 1)
    hht = [hhp.tile([P, T], F16, name=f"hh_{j}", tag="hh", bufs=JFF) for j in range(JFF)]

    ln_psum = pool("ln_psum3", 1, space="PSUM")
    ln3p = pool("ln3p", 1)
    ln3t = [ln3p.tile([P, T], F16, name=f"ln3_{c}", tag="ln3", bufs=KC) for c in range(KC)]
    layernorm(x2, T, 2, ln3t, ln_psum)
    close("ln_psum3")

    wpool = pool("wpool4a", 1)
    proj_psum = pool("proj_psum4", 1, space="PSUM")
    for j in range(JFF):
        wg = wpool.tile([P, KC, P], F16, name=f"wg_{j}", tag="wff1g", bufs=3)
        nc.sync.dma_start(wg[:], d["wff1"][JFF + j])
        gps = proj_psum.tile([P, 512], F32, name=f"gps_{j}", tag="proj", bufs=4)
        for kc in range(KC):
            nc.tensor.matmul(gps[:], wg[:, kc], ln3t[kc][:], start=(kc == 0),
                             stop=(kc == KC - 1))
        gel = tmp.tile([P, T], F16, name=f"gel_{j}", tag="gel", bufs=3)
        if trivial_bias:
            nc.scalar.activation(gel[:], gps[:], AF.Gelu_apprx_tanh)
        else:
            nc.scalar.activation(gel[:], gps[:], AF.Gelu_apprx_tanh, bias=bias_ap(60 + j))

        wa = wpool.tile([P, KC, P], F16, name=f"wa_{j}", tag="wff1a", bufs=3)
        nc.sync.dma_start(wa[:], d["wff1"][j])
        aps = proj_psum.tile([P, 512], F32, name=f"aps_{j}", tag="proj", bufs=4)
        for kc in range(KC):
            nc.tensor.matmul(aps[:], wa[:, kc], ln3t[kc][:], start=(kc == 0),
                             stop=(kc == KC - 1))
        if trivial_bias:
            nc.vector.tensor_mul(hht[j][:], aps[:], gel[:])
        else:
            nc.vector.scalar_tensor_tensor(hht[j][:], aps[:], bias_ap(20 + j), gel[:],
                                           mybir.AluOpType.add, mybir.AluOpType.mult)

    close("wpool4a", "ln3p")

    # ---------------- phase 9: FF down-proj + residual -> out ----------------

    wpool = pool("wpool4b", 1)
    outp = pool("outp", 4)
    for mc in range(KC):
        wt = wpool.tile([P, JFF, P], F16, name=f"wff2_{mc}", tag="wff2", bufs=2)
        nc.sync.dma_start(wt[:], d["wff2"][mc])
        ps = proj_psum.tile([P, 512], F32, name=f"psf2_{mc}", tag="proj", bufs=4)
        for kc in range(JFF):
            nc.tensor.matmul(ps[:], wt[:, kc], hht[kc][:], start=(kc == 0),
                             stop=(kc == JFF - 1))
        ot = outp.tile([P, T], F32, name=f"out_{mc}", tag="out")
        if trivial_bias:
            nc.vector.tensor_add(ot[:], ps[:], x2[mc][:])
        else:
            nc.vector.scalar_tensor_tensor(ot[:], ps[:], bias_ap(100 + mc), x2[mc][:],
                                           mybir.AluOpType.add, mybir.AluOpType.add)
        nc.sync.dma_start(d["out"][mc * P:(mc + 1) * P, :], ot[:])

    close("outp", "wpool4b", "hhp", "x2p", "o2p", "tmp", "const", "proj_psum4")


def _lhst_layout(w, n_kc, n_mc):
    """[K, M] f32 -> fp16 [n_mc, 128, n_kc, 128] so block [mc] is the
    contiguous stationary-operand group for output chunk mc."""
    return np.ascontiguousarray(
        w.reshape(n_kc, P, n_mc, P).transpose(2, 1, 0, 3).astype(np.float16))


def _rhs_layout(w, n_kc):
    """[K, M] f32 -> fp16 [n_kc, 128, M] row-chunk (moving-operand) layout."""
    return np.ascontiguousarray(w.reshape(n_kc, P, -1).astype(np.float16))


_BUILT = {}


def _build(trivial_aff, trivial_bias):
    key = (trivial_aff, trivial_bias)
    if key in _BUILT:
        return _BUILT[key]
    nc = bacc.Bacc("TRN2", target_bir_lowering=False, debug=False, num_devices=N_CORES)
    d = {
        "xt": nc.dram_tensor("xt", [DIM, TKV], F32, kind="ExternalInput").ap(),
        "ctxt": nc.dram_tensor("ctxt", [CTX_DIM, MCTX], F32, kind="ExternalInput").ap(),
        "wq1": nc.dram_tensor("wq1", [KC, P, KC, P], F16, kind="ExternalInput").ap(),
        "wk1": nc.dram_tensor("wk1", [KC, P, KC, P], F16, kind="ExternalInput").ap(),
        "wv1": nc.dram_tensor("wv1", [KC, P, DIM], F16, kind="ExternalInput").ap(),
        "wo1": nc.dram_tensor("wo1", [KC, P, KC, P], F16, kind="ExternalInput").ap(),
        "wq2": nc.dram_tensor("wq2", [KC, P, KC, P], F16, kind="ExternalInput").ap(),
        "wk2": nc.dram_tensor("wk2", [KC, P, KCX, P], F16, kind="ExternalInput").ap(),
        "wv2": nc.dram_tensor("wv2", [KCX, P, DIM], F16, kind="ExternalInput").ap(),
        "wo2": nc.dram_tensor("wo2", [KC, P, KC, P], F16, kind="ExternalInput").ap(),
        "wff1": nc.dram_tensor("wff1", [2 * JFF, P, KC, P], F16, kind="ExternalInput").ap(),
        "wff2": nc.dram_tensor("wff2", [KC, P, JFF, P], F16, kind="ExternalInput").ap(),
        "out": nc.dram_tensor("out", [DIM, T], F32, kind="ExternalOutput").ap(),
    }
    if not trivial_aff:
        d["aff"] = nc.dram_tensor("aff", [P, 60], F32, kind="ExternalInput").ap()
    if not trivial_bias:
        d["biases"] = nc.dram_tensor("biases", [P, 110], F32, kind="ExternalInput").ap()
    with tile.TileContext(nc) as tc:
        _emit(tc, d, trivial_aff, trivial_bias)
    nc.compile()
    _BUILT[key] = nc
    return nc


def kernel(x, context,
           g1, be1, wq1, wk1, wv1, wo1, bo1,
           g2, be2, wq2, wk2, wv2, wo2, bo2,
           g3, be3, w_ff1, b_ff1, w_ff2, b_ff2,
           _trace=False):
    global last_exec_time_ns
    x = np.asarray(x, np.float32)
    context = np.asarray(context, np.float32)

    affs = [np.asarray(a, np.float32) for a in (g1, be1, g2, be2, g3, be3)]
    biases = [np.asarray(b, np.float32) for b in (bo1, bo2, b_ff1, b_ff2)]
    trivial_aff = all(np.all(a == (1.0 if i % 2 == 0 else 0.0))
                      for i, a in enumerate(affs))
    trivial_bias = all(np.all(b == 0.0) for b in biases)

    nc = _build(trivial_aff, trivial_bias)

    shared = {
        "wq1": _lhst_layout(np.asarray(wq1, np.float32), KC, KC),
        "wk1": _lhst_layout(np.asarray(wk1, np.float32), KC, KC),
        "wv1": _rhs_layout(np.asarray(wv1, np.float32), KC),
        "wo1": _lhst_layout(np.asarray(wo1, np.float32), KC, KC),
        "wq2": _lhst_layout(np.asarray(wq2, np.float32), KC, KC),
        "wk2": _lhst_layout(np.asarray(wk2, np.float32), KCX, KC),
        "wv2": _rhs_layout(np.asarray(wv2, np.float32), KCX),
        "wo2": _lhst_layout(np.asarray(wo2, np.float32), KC, KC),
        "wff1": _lhst_layout(np.asarray(w_ff1, np.float32), KC, 2 * JFF),
        "wff2": _lhst_layout(np.asarray(w_ff2, np.float32), JFF, KC),
    }
    if not trivial_aff:
        aff = np.zeros([P, 60], np.float32)
        for i, a in enumerate(affs):
            # col = ln_idx*20 + (0 for g / 10 for be) + chunk
            ln_idx, j = i // 2, i % 2
            aff[:, ln_idx * 20 + j * 10: ln_idx * 20 + j * 10 + 10] = \
                a.reshape(KC, P).T
        shared["aff"] = aff
    if not trivial_bias:
        bb = np.zeros([P, 110], np.float32)
        bb[:, 0:10] = biases[0].reshape(KC, P).T
        bb[:, 10:20] = biases[1].reshape(KC, P).T
        bb[:, 20:100] = biases[2].reshape(2 * JFF, P).T
        bb[:, 100:110] = biases[3].reshape(KC, P).T
        shared["biases"] = bb

    in_maps = []
    for b in range(BATCH):
        ctxt = np.ascontiguousarray(context[b].T)
        for h in range(2):
            xr = np.roll(x[b], -h * T, axis=0)
            m = dict(shared)
            m["xt"] = np.ascontiguousarray(xr.T)
            m["ctxt"] = ctxt
            in_maps.append(m)

    res = bass_utils.run_bass_kernel_spmd(
        nc, in_maps, core_ids=list(range(N_CORES)), trace=_trace)
    last_exec_time_ns = res.exec_time_ns

    out = np.empty((BATCH, NTOK, DIM), np.float32)
    for b in range(BATCH):
        for h in range(2):
            out[b, h * T:(h + 1) * T, :] = res.results[b * 2 + h]["out"].T
    return out


# revision 10
# speedup vs baseline: 1.0803x; 1.0803x over previous
"""BasicTransformerBlock on 8 TRN2 NeuronCores.

Sharding: data-parallel, core = (batch b in 0..3) x (sequence half h in 0..1).
Each core receives its batch element's full sequence rotated so its local 512
rows come first (softmax over keys is permutation invariant), computes K/V of
attn1 for all 1024 tokens (duplicated across the pair, ~10% extra FLOPs, zero
collectives), and everything else for its 512 local tokens only.

On-chip layout: feature-major activations [features on partitions, tokens on
free axis] so every projection consumes natural-layout weights as the matmul
stationary operand. Matmuls run in fp16 (weights pre-cast on host); the
residual stream, LN math and PSUM accumulation stay fp32. LayerNorm partition
reductions and per-token broadcasts use float32r ones-matmuls. Attention
softmax denominators come free from a ones-column appended to V.
"""

import sys
import types

sys.path.insert(0, "/opt/trn_rl_repo")

# concourse fetches the NTFF profile hook from antenv.axon_hooks, which the
# agent image's antenv stub lacks. Register a shim so trace=True works.
if "antenv.axon_hooks" not in sys.modules:
    _hooks = types.ModuleType("antenv.axon_hooks")
    _HOOK = [None]

    def _get_hook():
        if _HOOK[0] is None:
            try:
                from trn_agent_boot.trn_boot import _ntff_profile_via_ctypes

                _HOOK[0] = _ntff_profile_via_ctypes("/opt/axon/libaxon_pjrt.so")
            except Exception:
                _HOOK[0] = None
        return _HOOK[0]

    _hooks.get_axon_ntff_profile_hook = _get_hook
    _hooks.set_axon_ntff_profile_hook = lambda h: _HOOK.__setitem__(0, h)
    sys.modules["antenv.axon_hooks"] = _hooks
    try:
        import antenv

        antenv.axon_hooks = _hooks
    except ImportError:
        pass

import numpy as np

import concourse.bass as bass
import concourse.mybir as mybir
import concourse.tile as tile
from concourse import bacc, bass_utils

dt = mybir.dt
F32, F16, F32R = dt.float32, dt.float16, dt.float32r
AF = mybir.ActivationFunctionType

DIM, HEADS, DHEAD, CTX_DIM, DFF = 1280, 20, 64, 768, 5120
BATCH, NTOK, MCTX = 4, 1024, 77
EPS = 1e-5
SCALE = DHEAD ** -0.5
N_CORES = 8
T = 512         # local tokens per core
TKV = 1024      # attn1 key/value tokens per core
KC = DIM // 128           # 10
KCX = CTX_DIM // 128      # 6
JFF = DFF // 128          # 40 (chunks of the gated hidden)
P = 128

last_exec_time_ns = None


def _emit(tc, d, trivial_aff, trivial_bias):
    nc = tc.nc
    pools = {}

    def pool(name, bufs, space="SBUF", side="left"):
        p = tc.alloc_tile_pool(name=name, bufs=bufs, space=space, side=side)
        pools[name] = p
        return p

    def close(*names):
        for n in names:
            pools.pop(n).release()

    # Pools are two LIFO stacks (left/right) per memory space; lifetimes below
    # are arranged so every release pops the top of its stack.
    const = pool("const", 1)
    ones_col = const.tile([P, 1], F16, name="ones_col")
    nc.vector.memset(ones_col[:], 1.0)
    ones_row = const.tile([1, P], F16, name="ones_row")
    nc.vector.memset(ones_row[:], 1.0)
    if not trivial_aff:
        aff = const.tile([P, 60], F32, name="aff")
        nc.sync.dma_start(aff[:], d["aff"])
    if not trivial_bias:
        biases = const.tile([P, 110], F32, name="biases")
        nc.sync.dma_start(biases[:], d["biases"])

    tmp = pool("tmp", 1)

    # ---------------- helpers ----------------

    def layernorm(x_tiles, Ttok, ln_idx, out_tiles, ln_psum):
        """x_tiles: KC SBUF tiles [128, Ttok] f32 -> out_tiles [128, Ttok] fp16."""
        for t in range(Ttok // 512):
            sl = slice(t * 512, (t + 1) * 512)
            sums_ps = ln_psum.tile([1, 512], F32, name=f"lns{ln_idx}_{t}", tag="lnstat", bufs=2)
            sq_ps = ln_psum.tile([1, 512], F32, name=f"lnq{ln_idx}_{t}", tag="lnstat", bufs=2)
            for c in range(KC):
                xh = tmp.tile([P, 512], F16, name=f"xh{ln_idx}_{t}_{c}", tag="xh", bufs=3)
                nc.any.tensor_copy(out=xh[:], in_=x_tiles[c][:, sl])
                xsq = tmp.tile([P, 512], F16, name=f"xsq{ln_idx}_{t}_{c}", tag="xsq", bufs=3)
                nc.vector.tensor_mul(xsq[:], x_tiles[c][:, sl], x_tiles[c][:, sl])
                nc.tensor.matmul(sums_ps[:], ones_col[:], xh[:],
                                 start=(c == 0), stop=(c == KC - 1))
                nc.tensor.matmul(sq_ps[:], ones_col[:], xsq[:],
                                 start=(c == 0), stop=(c == KC - 1))
            ssum = tmp.tile([1, 512], F16, name=f"ssum{ln_idx}_{t}", tag="ssum", bufs=2)
            nc.scalar.copy(ssum[:], sums_ps[:])
            ssq = tmp.tile([1, 512], F16, name=f"ssq{ln_idx}_{t}", tag="ssq", bufs=2)
            nc.scalar.copy(ssq[:], sq_ps[:])
            bs_ps = ln_psum.tile([P, 512], F32, name=f"bs{ln_idx}_{t}", tag="lnbc", bufs=2)
            nc.tensor.matmul(bs_ps[:], ones_row[:], ssum[:], start=True, stop=True)
            bq_ps = ln_psum.tile([P, 512], F32, name=f"bq{ln_idx}_{t}", tag="lnbc", bufs=2)
            nc.tensor.matmul(bq_ps[:], ones_row[:], ssq[:], start=True, stop=True)
            mu = tmp.tile([P, 512], F32, name=f"mu{ln_idx}_{t}", tag="mu", bufs=2)
            nc.vector.tensor_scalar_mul(mu[:], bs_ps[:], 1.0 / DIM)
            musq = tmp.tile([P, 512], F32, name=f"musq{ln_idx}_{t}", tag="musq", bufs=2)
            nc.vector.tensor_mul(musq[:], mu[:], mu[:])
            # musq - EPS, so var = ex2 - musq + EPS below
            nc.vector.tensor_scalar_sub(musq[:], musq[:], EPS)
            var = tmp.tile([P, 512], F32, name=f"var{ln_idx}_{t}", tag="var", bufs=2)
            nc.vector.scalar_tensor_tensor(var[:], bq_ps[:], 1.0 / DIM, musq[:],
                                           mybir.AluOpType.mult, mybir.AluOpType.subtract)
            std = tmp.tile([P, 512], F32, name=f"std{ln_idx}_{t}", tag="std", bufs=2)
            nc.scalar.sqrt(std[:], var[:])
            rstd = tmp.tile([P, 512], F32, name=f"rstd{ln_idx}_{t}", tag="rstd", bufs=2)
            nc.vector.reciprocal_approx_fast(rstd[:], std[:])
            for c in range(KC):
                xm = tmp.tile([P, 512], F32, name=f"xm{ln_idx}_{t}_{c}", tag="xm", bufs=3)
                nc.vector.tensor_sub(xm[:], x_tiles[c][:, sl], mu[:])
                if trivial_aff:
                    nc.vector.tensor_mul(out_tiles[c][:, sl], xm[:], rstd[:])
                else:
                    xn = tmp.tile([P, 512], F32, name=f"xn{ln_idx}_{t}_{c}", tag="xn", bufs=3)
                    nc.vector.tensor_mul(xn[:], xm[:], rstd[:])
                    g_ap = aff[:, ln_idx * 20 + c: ln_idx * 20 + c + 1]
                    be_ap = aff[:, ln_idx * 20 + 10 + c: ln_idx * 20 + 10 + c + 1]
                    xg = tmp.tile([P, 512], F32, name=f"xg{ln_idx}_{t}_{c}", tag="xg", bufs=3)
                    nc.vector.tensor_scalar_mul(xg[:], xn[:], g_ap)
                    nc.scalar.activation(out_tiles[c][:, sl], xg[:], AF.Copy, bias=be_ap)

    def attn_scores(c, h, Kt_c, Qt_c, n_kc, kv_par, sc_psum, epool):
        """Scores + exp for one head (PE + ACT stage of the pipeline)."""
        head = 2 * c + h
        hp = slice(64 * h, 64 * h + 64)
        exps = []
        for k8 in range(n_kc):
            sps = sc_psum.tile([kv_par, 512], F32, name=f"sps{head}_{k8}", tag="sc", bufs=4)
            nc.tensor.matmul(sps[:], Kt_c[hp, k8 * kv_par:(k8 + 1) * kv_par], Qt_c[hp, :],
                             start=True, stop=True, tile_position=(64 * h, 0))
            e = epool.tile([kv_par, 512], F16, name=f"exp{head}_{k8}", tag="exp")
            nc.scalar.activation(e[:], sps[:], AF.Exp, scale=SCALE)
            exps.append(e)
        return exps

    def attn_v(head, exps, Vt, kv_par, ov_psum):
        ops_ = ov_psum.tile([DHEAD + 1, 512], F32, name=f"ov{head}", tag="ov", bufs=2)
        for k8 in range(len(exps)):
            nc.tensor.matmul(ops_[:], Vt[k8][:kv_par, head, :], exps[k8][:],
                             start=(k8 == 0), stop=(k8 == len(exps) - 1))
        return ops_

    def attn_finish(head, ops_, ov_psum, out_ap):
        den = tmp.tile([1, 512], F32, name=f"den{head}", tag="den", bufs=3)
        nc.scalar.copy(den[:], ops_[DHEAD:DHEAD + 1, :])
        rec32 = tmp.tile([1, 512], F32, name=f"rec32_{head}", tag="rec32", bufs=3)
        nc.vector.reciprocal_approx_fast(rec32[:], den[:])
        rec = tmp.tile([1, 512], F16, name=f"rec{head}", tag="rec", bufs=3)
        nc.vector.tensor_copy(out=rec[:], in_=rec32[:])
        bps = ov_psum.tile([DHEAD, 512], F32, name=f"bps{head}", tag="obc", bufs=2)
        nc.tensor.matmul(bps[:], ones_row[:, :DHEAD], rec[:],
                         start=True, stop=True)
        bsb = tmp.tile([DHEAD, 512], F32, name=f"bsb{head}", tag="bsb", bufs=3)
        nc.vector.tensor_copy(out=bsb[:], in_=bps[:])
        nc.vector.tensor_mul(out_ap, ops_[:DHEAD, :], bsb[:])

    def attn_pipeline(Kt_, Qt_, Vt_, n_kc, kv_par, sc_psum, ov_psum, epool, O_out):
        """Pairs pipelined one deep: scores/exp of pair c+1 are issued before
        attnV of pair c, so the PE streams while ACT computes exps."""
        prev = None
        for c in range(KC):
            e0 = attn_scores(c, 0, Kt_[c], Qt_[c], n_kc, kv_par, sc_psum, epool)
            e1 = attn_scores(c, 1, Kt_[c], Qt_[c], n_kc, kv_par, sc_psum, epool)
            if prev is not None:
                pc, pe0, pe1 = prev
                o0 = attn_v(2 * pc, pe0, Vt_, kv_par, ov_psum)
                o1 = attn_v(2 * pc + 1, pe1, Vt_, kv_par, ov_psum)
                attn_finish(2 * pc, o0, ov_psum, O_out[pc][0:DHEAD, :])
                attn_finish(2 * pc + 1, o1, ov_psum, O_out[pc][DHEAD:2 * DHEAD, :])
            prev = (c, e0, e1)
        pc, pe0, pe1 = prev
        o0 = attn_v(2 * pc, pe0, Vt_, kv_par, ov_psum)
        o1 = attn_v(2 * pc + 1, pe1, Vt_, kv_par, ov_psum)
        attn_finish(2 * pc, o0, ov_psum, O_out[pc][0:DHEAD, :])
        attn_finish(2 * pc + 1, o1, ov_psum, O_out[pc][DHEAD:2 * DHEAD, :])

    def project(w_d, n_kc, rhs_fn, n_mc, consume, wpool, wtag, psum_p, wbufs=3):
        """out[mc] = sum_kc w[mc][:, kc].T @ rhs(kc); consume(mc, psum)."""
        for mc in range(n_mc):
            wt = wpool.tile([P, n_kc, P], F16, name=f"{wtag}_{mc}", tag=wtag, bufs=wbufs)
            nc.sync.dma_start(wt[:], w_d[mc])
            ps = psum_p.tile([P, 512], F32, name=f"ps_{wtag}_{mc}", tag="proj", bufs=4)
            for kc in range(n_kc):
                nc.tensor.matmul(ps[:], wt[:, kc], rhs_fn(kc),
                                 start=(kc == 0), stop=(kc == n_kc - 1))
            consume(mc, ps)

    def bias_ap(col):
        return biases[:, col:col + 1]

    # ---------------- phase 1: load x, LN1 ----------------

    resp = pool("resp", 1)
    ln1p = pool("ln1p", 1)
    ln1t = [ln1p.tile([P, TKV], F16, name=f"ln1_{c}", tag="ln1", bufs=KC) for c in range(KC)]

    ln_psum = pool("ln_psum", 1, space="PSUM")
    xpool = pool("xpool", 1, side="right")
    x_sb = []
    for c in range(KC):
        xc = xpool.tile([P, TKV], F32, name=f"x_{c}", tag="x", bufs=KC)
        nc.sync.dma_start(xc[:], d["xt"][c * P:(c + 1) * P, :])
        x_sb.append(xc)
    resid = []
    for c in range(KC):
        rc = resp.tile([P, T], F32, name=f"res_{c}", tag="res", bufs=KC)
        nc.scalar.copy(rc[:], x_sb[c][:, 0:T])
        resid.append(rc)
    layernorm(x_sb, TKV, 0, ln1t, ln_psum)
    close("xpool", "ln_psum")

    # ---------------- phase 2: Q, K, V projections ----------------

    proj_psum = pool("proj_psum", 1, space="PSUM")
    wpool = pool("wpool1", 1)
    qkv = pool("qkv", 1, side="right")

    Qt = [qkv.tile([P, T], F16, name=f"qt_{mc}", tag="qt", bufs=KC) for mc in range(KC)]
    Kt = [qkv.tile([P, TKV], F16, name=f"kt_{mc}", tag="kt", bufs=KC) for mc in range(KC)]
    Vt = [qkv.tile([P, HEADS, DHEAD + 1], F16, name=f"vt_{t8}", tag="vt", bufs=8)
          for t8 in range(8)]

    def q_consume(mc, ps):
        nc.any.tensor_copy(out=Qt[mc][:], in_=ps[:])

    project(d["wq1"], KC, lambda kc: ln1t[kc][:, 0:T], KC, q_consume, wpool, "wq1", proj_psum)

    for thalf in range(2):
        sl = slice(thalf * 512, (thalf + 1) * 512)

        def k_consume(mc, ps, sl=sl):
            nc.any.tensor_copy(out=Kt[mc][:, sl], in_=ps[:])

        project(d["wk1"], KC, lambda kc, sl=sl: ln1t[kc][:, sl], KC, k_consume, wpool, "wk1",
                proj_psum)

    # V token-major with a ones column per head (softmax denominator trick)
    wv_sb = []
    for kc in range(KC):
        wv = wpool.tile([P, DIM], F16, name=f"wv1_{kc}", tag="wv1", bufs=KC)
        nc.sync.dma_start(wv[:], d["wv1"][kc])
        wv_sb.append(wv)
    for t8 in range(8):
        nc.vector.memset(Vt[t8][:], 1.0)
        for n0, nsz in ((0, 512), (512, 512), (1024, 256)):
            ps = proj_psum.tile([P, 512], F32, name=f"psv_{t8}_{n0}", tag="proj", bufs=4)
            for kc in range(KC):
                nc.tensor.matmul(ps[:, :nsz], ln1t[kc][:, t8 * P:(t8 + 1) * P],
                                 wv_sb[kc][:, n0:n0 + nsz],
                                 start=(kc == 0), stop=(kc == KC - 1))
            nc.any.tensor_copy(
                out=Vt[t8][:, n0 // DHEAD:(n0 + nsz) // DHEAD, 0:DHEAD],
                in_=ps[:, :nsz].rearrange("p (h e) -> p h e", e=DHEAD))

    close("wpool1", "ln1p", "proj_psum")

    # ---------------- phase 3: attn1 ----------------

    otp = pool("otp", 1)
    Ot = [otp.tile([P, T], F16, name=f"ot_{c}", tag="ot", bufs=KC) for c in range(KC)]
    sc_psum = pool("sc_psum", 1, space="PSUM")
    ov_psum = pool("ov_psum", 1, space="PSUM")
    epool = pool("epool", 32, side="right")

    attn_pipeline(Kt, Qt, Vt, 8, P, sc_psum, ov_psum, epool, Ot)

    close("epool", "qkv", "ov_psum", "sc_psum")

    # ---------------- phase 4: out-proj 1 + residual ----------------

    wpool = pool("wpool2", 1)
    proj_psum = pool("proj_psum2", 1, space="PSUM")
    x1p = pool("x1p", 1, side="right")
    x1 = [x1p.tile([P, T], F32, name=f"x1_{mc}", tag="x1", bufs=KC) for mc in range(KC)]

    def o1_consume(mc, ps):
        if trivial_bias:
            nc.vector.tensor_add(x1[mc][:], ps[:], resid[mc][:])
        else:
            nc.vector.scalar_tensor_tensor(x1[mc][:], ps[:], bias_ap(mc), resid[mc][:],
                                           mybir.AluOpType.add, mybir.AluOpType.add)

    project(d["wo1"], KC, lambda kc: Ot[kc][:], KC, o1_consume, wpool, "wo1", proj_psum)
    close("wpool2", "otp", "resp", "proj_psum2")

    # ---------------- phase 5: LN2 + attn2 projections ----------------

    ln_psum = pool("ln_psum2", 1, space="PSUM")
    ln2p = pool("ln2p", 1)
    ln2t = [ln2p.tile([P, T], F16, name=f"ln2_{c}", tag="ln2", bufs=KC) for c in range(KC)]
    layernorm(x1, T, 1, ln2t, ln_psum)
    close("ln_psum2")

    proj_psum = pool("proj_psum2b", 1, space="PSUM")
    qkv2 = pool("qkv2", 1, side="right")
    wpool = pool("wpool2b", 1)
    ctx_sb = []
    for c in range(KCX):
        cc = qkv2.tile([P, MCTX], F32, name=f"ctx_{c}", tag="ctx", bufs=KCX)
        nc.sync.dma_start(cc[:], d["ctxt"][c * P:(c + 1) * P, :])
        ch = qkv2.tile([P, MCTX], F16, name=f"ctxh_{c}", tag="ctxh", bufs=KCX)
        nc.any.tensor_copy(out=ch[:], in_=cc[:])
        ctx_sb.append(ch)

    Q2t = [qkv2.tile([P, T], F16, name=f"q2t_{mc}", tag="q2t", bufs=KC) for mc in range(KC)]
    K2t = [qkv2.tile([P, MCTX], F16, name=f"k2t_{mc}", tag="k2t", bufs=KC) for mc in range(KC)]
    V2t = [qkv2.tile([P, HEADS, DHEAD + 1], F16, name="v2t", tag="v2t", bufs=1)]

    def q2_consume(mc, ps):
        nc.any.tensor_copy(out=Q2t[mc][:], in_=ps[:])

    project(d["wq2"], KC, lambda kc: ln2t[kc][:], KC, q2_consume, wpool, "wq2", proj_psum)

    for mc in range(KC):
        wt = wpool.tile([P, KCX, P], F16, name=f"wk2_{mc}", tag="wk2", bufs=3)
        nc.sync.dma_start(wt[:], d["wk2"][mc])
        ps = proj_psum.tile([P, MCTX], F32, name=f"psk2_{mc}", tag="projx", bufs=2)
        for kc in range(KCX):
            nc.tensor.matmul(ps[:], wt[:, kc], ctx_sb[kc][:], start=(kc == 0),
                             stop=(kc == KCX - 1))
        nc.any.tensor_copy(out=K2t[mc][:], in_=ps[:])

    wv2_sb = []
    for kc in range(KCX):
        wv = wpool.tile([P, DIM], F16, name=f"wv2_{kc}", tag="wv2", bufs=KCX)
        nc.sync.dma_start(wv[:], d["wv2"][kc])
        wv2_sb.append(wv)
    nc.vector.memset(V2t[0][:], 1.0)
    for n0, nsz in ((0, 512), (512, 512), (1024, 256)):
        ps = proj_psum.tile([MCTX, 512], F32, name=f"psv2_{n0}", tag="proj", bufs=4)
        for kc in range(KCX):
            nc.tensor.matmul(ps[:, :nsz], ctx_sb[kc][:], wv2_sb[kc][:, n0:n0 + nsz],
                             start=(kc == 0), stop=(kc == KCX - 1))
        nc.any.tensor_copy(
            out=V2t[0][:MCTX, n0 // DHEAD:(n0 + nsz) // DHEAD, 0:DHEAD],
            in_=ps[:, :nsz].rearrange("p (h e) -> p h e", e=DHEAD))

    close("wpool2b", "ln2p", "proj_psum2b")

    # ---------------- phase 6: attn2 ----------------

    o2p = pool("o2p", 1)
    O2t = [o2p.tile([P, T], F16, name=f"o2t_{c}", tag="o2t", bufs=KC) for c in range(KC)]
    sc_psum = pool("sc_psum2", 1, space="PSUM")
    ov_psum = pool("ov_psum2", 1, space="PSUM")
    epool = pool("epool2", 6, side="right")

    attn_pipeline(K2t, Q2t, V2t, 1, MCTX, sc_psum, ov_psum, epool, O2t)

    close("epool2", "qkv2", "ov_psum2", "sc_psum2")

    # ---------------- phase 7: out-proj 2 + residual ----------------

    x2p = pool("x2p", 1)
    wpool = pool("wpool3", 1)
    proj_psum = pool("proj_psum3", 1, space="PSUM")
    x2 = [x2p.tile([P, T], F32, name=f"x2_{mc}", tag="x2", bufs=KC) for mc in range(KC)]

    def o2_consume(mc, ps):
        if trivial_bias:
            nc.vector.tensor_add(x2[mc][:], ps[:], x1[mc][:])
        else:
            nc.vector.scalar_tensor_tensor(x2[mc][:], ps[:], bias_ap(10 + mc), x1[mc][:],
                                           mybir.AluOpType.add, mybir.AluOpType.add)

    project(d["wo2"], KC, lambda kc: O2t[kc][:], KC, o2_consume, wpool, "wo2", proj_psum)
    close("wpool3", "x1p", "proj_psum3")

    # ---------------- phase 8: LN3 + GEGLU FF ----------------

    hhp = pool("hhp", 1)
    hht = [hhp.tile([P, T], F16, name=f"hh_{j}", tag="hh", bufs=JFF) for j in range(JFF)]

    ln_psum = pool("ln_psum3", 1, space="PSUM")
    ln3p = pool("ln3p", 1)
    ln3t = [ln3p.tile([P, T], F16, name=f"ln3_{c}", tag="ln3", bufs=KC) for c in range(KC)]
    layernorm(x2, T, 2, ln3t, ln_psum)
    close("ln_psum3")

    wpool = pool("wpool4a", 1)
    proj_psum = pool("proj_psum4", 1, space="PSUM")
    for j in range(JFF):
        wg = wpool.tile([P, KC, P], F16, name=f"wg_{j}", tag="wff1g", bufs=3)
        nc.sync.dma_start(wg[:], d["wff1"][JFF + j])
        gps = proj_psum.tile([P, 512], F32, name=f"gps_{j}", tag="proj", bufs=4)
        for kc in range(KC):
            nc.tensor.matmul(gps[:], wg[:, kc], ln3t[kc][:], start=(kc == 0),
                             stop=(kc == KC - 1))
        gel = tmp.tile([P, T], F16, name=f"gel_{j}", tag="gel", bufs=3)
        if trivial_bias:
            nc.scalar.activation(gel[:], gps[:], AF.Gelu_apprx_tanh)
        else:
            nc.scalar.activation(gel[:], gps[:], AF.Gelu_apprx_tanh, bias=bias_ap(60 + j))

        wa = wpool.tile([P, KC, P], F16, name=f"wa_{j}", tag="wff1a", bufs=3)
        nc.sync.dma_start(wa[:], d["wff1"][j])
        aps = proj_psum.tile([P, 512], F32, name=f"aps_{j}", tag="proj", bufs=4)
        for kc in range(KC):
            nc.tensor.matmul(aps[:], wa[:, kc], ln3t[kc][:], start=(kc == 0),
                             stop=(kc == KC - 1))
        if trivial_bias:
            nc.vector.tensor_mul(hht[j][:], aps[:], gel[:])
        else:
            nc.vector.scalar_tensor_tensor(hht[j][:], aps[:], bias_ap(20 + j), gel[:],
                                           mybir.AluOpType.add, mybir.AluOpType.mult)

    close("wpool4a", "ln3p")

    # ---------------- phase 9: FF down-proj + residual -> out ----------------

    wpool = pool("wpool4b", 1)
    outp = pool("outp", 4)
    for mc in range(KC):
        wt = wpool.tile([P, JFF, P], F16, name=f"wff2_{mc}", tag="wff2", bufs=2)
        nc.sync.dma_start(wt[:], d["wff2"][mc])
        ps = proj_psum.tile([P, 512], F32, name=f"psf2_{mc}", tag="proj", bufs=4)
        for kc in range(JFF):
            nc.tensor.matmul(ps[:], wt[:, kc], hht[kc][:], start=(kc == 0),
                             stop=(kc == JFF - 1))
        ot = outp.tile([P, T], F32, name=f"out_{mc}", tag="out")
        if trivial_bias:
            nc.vector.tensor_add(ot[:], ps[:], x2[mc][:])
        else:
            nc.vector.scalar_tensor_tensor(ot[:], ps[:], bias_ap(100 + mc), x2[mc][:],
                                           mybir.AluOpType.add, mybir.AluOpType.add)
        nc.sync.dma_start(d["out"][mc * P:(mc + 1) * P, :], ot[:])

    close("outp", "wpool4b", "hhp", "x2p", "o2p", "tmp", "const", "proj_psum4")


def _lhst_layout(w, n_kc, n_mc):
    """[K, M] f32 -> fp16 [n_mc, 128, n_kc, 128] so block [mc] is the
    contiguous stationary-operand group for output chunk mc."""
    return np.ascontiguousarray(
        w.reshape(n_kc, P, n_mc, P).transpose(2, 1, 0, 3).astype(np.float16))


def _rhs_layout(w, n_kc):
    """[K, M] f32 -> fp16 [n_kc, 128, M] row-chunk (moving-operand) layout."""
    return np.ascontiguousarray(w.reshape(n_kc, P, -1).astype(np.float16))


_BUILT = {}


def _build(trivial_aff, trivial_bias):
    key = (trivial_aff, trivial_bias)
    if key in _BUILT:
        return _BUILT[key]
    nc = bacc.Bacc("TRN2", target_bir_lowering=False, debug=False, num_devices=N_CORES)
    d = {
        "xt": nc.dram_tensor("xt", [DIM, TKV], F32, kind="ExternalInput").ap(),
        "ctxt": nc.dram_tensor("ctxt", [CTX_DIM, MCTX], F32, kind="ExternalInput").ap(),
        "wq1": nc.dram_tensor("wq1", [KC, P, KC, P], F16, kind="ExternalInput").ap(),
        "wk1": nc.dram_tensor("wk1", [KC, P, KC, P], F16, kind="ExternalInput").ap(),
        "wv1": nc.dram_tensor("wv1", [KC, P, DIM], F16, kind="ExternalInput").ap(),
        "wo1": nc.dram_tensor("wo1", [KC, P, KC, P], F16, kind="ExternalInput").ap(),
        "wq2": nc.dram_tensor("wq2", [KC, P, KC, P], F16, kind="ExternalInput").ap(),
        "wk2": nc.dram_tensor("wk2", [KC, P, KCX, P], F16, kind="ExternalInput").ap(),
        "wv2": nc.dram_tensor("wv2", [KCX, P, DIM], F16, kind="ExternalInput").ap(),
        "wo2": nc.dram_tensor("wo2", [KC, P, KC, P], F16, kind="ExternalInput").ap(),
        "wff1": nc.dram_tensor("wff1", [2 * JFF, P, KC, P], F16, kind="ExternalInput").ap(),
        "wff2": nc.dram_tensor("wff2", [KC, P, JFF, P], F16, kind="ExternalInput").ap(),
        "out": nc.dram_tensor("out", [DIM, T], F32, kind="ExternalOutput").ap(),
    }
    if not trivial_aff:
        d["aff"] = nc.dram_tensor("aff", [P, 60], F32, kind="ExternalInput").ap()
    if not trivial_bias:
        d["biases"] = nc.dram_tensor("biases", [P, 110], F32, kind="ExternalInput").ap()
    with tile.TileContext(nc) as tc:
        _emit(tc, d, trivial_aff, trivial_bias)
    nc.compile()
    _BUILT[key] = nc
    return nc


def kernel(x, context,
           g1, be1, wq1, wk1, wv1, wo1, bo1,
           g2, be2, wq2, wk2, wv2, wo2, bo2,
           g3, be3, w_ff1, b_ff1, w_ff2, b_ff2,
           _trace=False):
    global last_exec_time_ns
    x = np.asarray(x, np.float32)
    context = np.asarray(context, np.float32)

    affs = [np.asarray(a, np.float32) for a in (g1, be1, g2, be2, g3, be3)]
    biases = [np.asarray(b, np.float32) for b in (bo1, bo2, b_ff1, b_ff2)]
    trivial_aff = all(np.all(a == (1.0 if i % 2 == 0 else 0.0))
                      for i, a in enumerate(affs))
    trivial_bias = all(np.all(b == 0.0) for b in biases)

    nc = _build(trivial_aff, trivial_bias)

    shared = {
        "wq1": _lhst_layout(np.asarray(wq1, np.float32), KC, KC),
        "wk1": _lhst_layout(np.asarray(wk1, np.float32), KC, KC),
        "wv1": _rhs_layout(np.asarray(wv1, np.float32), KC),
        "wo1": _lhst_layout(np.asarray(wo1, np.float32), KC, KC),
        "wq2": _lhst_layout(np.asarray(wq2, np.float32), KC, KC),
        "wk2": _lhst_layout(np.asarray(wk2, np.float32), KCX, KC),
        "wv2": _rhs_layout(np.asarray(wv2, np.float32), KCX),
        "wo2": _lhst_layout(np.asarray(wo2, np.float32), KC, KC),
        "wff1": _lhst_layout(np.asarray(w_ff1, np.float32), KC, 2 * JFF),
        "wff2": _lhst_layout(np.asarray(w_ff2, np.float32), JFF, KC),
    }
    if not trivial_aff:
        aff = np.zeros([P, 60], np.float32)
        for i, a in enumerate(affs):
            # col = ln_idx*20 + (0 for g / 10 for be) + chunk
            ln_idx, j = i // 2, i % 2
            aff[:, ln_idx * 20 + j * 10: ln_idx * 20 + j * 10 + 10] = \
                a.reshape(KC, P).T
        shared["aff"] = aff
    if not trivial_bias:
        bb = np.zeros([P, 110], np.float32)
        bb[:, 0:10] = biases[0].reshape(KC, P).T
        bb[:, 10:20] = biases[1].reshape(KC, P).T
        bb[:, 20:100] = biases[2].reshape(2 * JFF, P).T
        bb[:, 100:110] = biases[3].reshape(KC, P).T
        shared["biases"] = bb

    in_maps = []
    for b in range(BATCH):
        ctxt = np.ascontiguousarray(context[b].T)
        for h in range(2):
            xr = np.roll(x[b], -h * T, axis=0)
            m = dict(shared)
            m["xt"] = np.ascontiguousarray(xr.T)
            m["ctxt"] = ctxt
            in_maps.append(m)

    res = bass_utils.run_bass_kernel_spmd(
        nc, in_maps, core_ids=list(range(N_CORES)), trace=_trace)
    last_exec_time_ns = res.exec_time_ns

    out = np.empty((BATCH, NTOK, DIM), np.float32)
    for b in range(BATCH):
        for h in range(2):
            out[b, h * T:(h + 1) * T, :] = res.results[b * 2 + h]["out"].T
    return out


# revision 12
# speedup vs baseline: 1.1609x; 1.0746x over previous
"""BasicTransformerBlock on 8 TRN2 NeuronCores.

Sharding: data-parallel, core = (batch b in 0..3) x (sequence half h in 0..1).
Each core receives its batch element's full sequence rotated so its local 512
rows come first (softmax over keys is permutation invariant), computes K/V of
attn1 for all 1024 tokens (duplicated across the pair, ~10% extra FLOPs, zero
collectives), and everything else for its 512 local tokens only.

On-chip layout: feature-major activations [features on partitions, tokens on
free axis] so every projection consumes natural-layout weights as the matmul
stationary operand. Matmuls run in fp16 (weights pre-cast on host); the
residual stream, LN math and PSUM accumulation stay fp32. LayerNorm partition
reductions and per-token broadcasts use float32r ones-matmuls. Attention
softmax denominators come free from a ones-column appended to V.
"""

import sys
import types

sys.path.insert(0, "/opt/trn_rl_repo")

# concourse fetches the NTFF profile hook from antenv.axon_hooks, which the
# agent image's antenv stub lacks. Register a shim so trace=True works.
if "antenv.axon_hooks" not in sys.modules:
    _hooks = types.ModuleType("antenv.axon_hooks")
    _HOOK = [None]

    def _get_hook():
        if _HOOK[0] is None:
            try:
                from trn_agent_boot.trn_boot import _ntff_profile_via_ctypes

                _HOOK[0] = _ntff_profile_via_ctypes("/opt/axon/libaxon_pjrt.so")
            except Exception:
                _HOOK[0] = None
        return _HOOK[0]

    _hooks.get_axon_ntff_profile_hook = _get_hook
    _hooks.set_axon_ntff_profile_hook = lambda h: _HOOK.__setitem__(0, h)
    sys.modules["antenv.axon_hooks"] = _hooks
    try:
        import antenv

        antenv.axon_hooks = _hooks
    except ImportError:
        pass

import numpy as np

import concourse.bass as bass
import concourse.mybir as mybir
import concourse.tile as tile
from concourse import bacc, bass_utils

dt = mybir.dt
F32, F16, F32R = dt.float32, dt.float16, dt.float32r
AF = mybir.ActivationFunctionType

DIM, HEADS, DHEAD, CTX_DIM, DFF = 1280, 20, 64, 768, 5120
BATCH, NTOK, MCTX = 4, 1024, 77
EPS = 1e-5
SCALE = DHEAD ** -0.5
N_CORES = 8
T = 512         # local tokens per core
TKV = 1024      # attn1 key/value tokens per core
KC = DIM // 128           # 10
KCX = CTX_DIM // 128      # 6
JFF = DFF // 128          # 40 (chunks of the gated hidden)
P = 128

last_exec_time_ns = None


def _emit(tc, d, trivial_aff, trivial_bias):
    nc = tc.nc
    pools = {}

    def pool(name, bufs, space="SBUF", side="left"):
        p = tc.alloc_tile_pool(name=name, bufs=bufs, space=space, side=side)
        pools[name] = p
        return p

    def close(*names):
        for n in names:
            pools.pop(n).release()

    # Pools are two LIFO stacks (left/right) per memory space; lifetimes below
    # are arranged so every release pops the top of its stack.
    const = pool("const", 1)
    ones_col = const.tile([P, 1], F16, name="ones_col")
    nc.vector.memset(ones_col[:], 1.0)
    ones_row = const.tile([1, P], F16, name="ones_row")
    nc.vector.memset(ones_row[:], 1.0)
    if not trivial_aff:
        aff = const.tile([P, 60], F32, name="aff")
        nc.sync.dma_start(aff[:], d["aff"])
    if not trivial_bias:
        biases = const.tile([P, 110], F32, name="biases")
        nc.sync.dma_start(biases[:], d["biases"])

    tmp = pool("tmp", 1)

    # ---------------- helpers ----------------

    def layernorm(x_tiles, Ttok, ln_idx, out_tiles, ln_psum):
        """x_tiles: KC SBUF tiles [128, Ttok] f32 -> out_tiles [128, Ttok] fp16."""
        for t in range(Ttok // 512):
            sl = slice(t * 512, (t + 1) * 512)
            sums_ps = ln_psum.tile([1, 512], F32, name=f"lns{ln_idx}_{t}", tag="lnstat", bufs=2)
            sq_ps = ln_psum.tile([1, 512], F32, name=f"lnq{ln_idx}_{t}", tag="lnstat", bufs=2)
            for c in range(KC):
                xh = tmp.tile([P, 512], F16, name=f"xh{ln_idx}_{t}_{c}", tag="xh", bufs=3)
                nc.scalar.copy(xh[:], x_tiles[c][:, sl])
                xsq = tmp.tile([P, 512], F16, name=f"xsq{ln_idx}_{t}_{c}", tag="xsq", bufs=3)
                nc.vector.tensor_mul(xsq[:], x_tiles[c][:, sl], x_tiles[c][:, sl])
                nc.tensor.matmul(sums_ps[:], ones_col[:], xh[:],
                                 start=(c == 0), stop=(c == KC - 1))
                nc.tensor.matmul(sq_ps[:], ones_col[:], xsq[:],
                                 start=(c == 0), stop=(c == KC - 1))
            ssum = tmp.tile([1, 512], F16, name=f"ssum{ln_idx}_{t}", tag="ssum", bufs=2)
            nc.scalar.copy(ssum[:], sums_ps[:])
            ssq = tmp.tile([1, 512], F16, name=f"ssq{ln_idx}_{t}", tag="ssq", bufs=2)
            nc.scalar.copy(ssq[:], sq_ps[:])
            bs_ps = ln_psum.tile([P, 512], F32, name=f"bs{ln_idx}_{t}", tag="lnbc", bufs=2)
            nc.tensor.matmul(bs_ps[:], ones_row[:], ssum[:], start=True, stop=True)
            bq_ps = ln_psum.tile([P, 512], F32, name=f"bq{ln_idx}_{t}", tag="lnbc", bufs=2)
            nc.tensor.matmul(bq_ps[:], ones_row[:], ssq[:], start=True, stop=True)
            mu = tmp.tile([P, 512], F32, name=f"mu{ln_idx}_{t}", tag="mu", bufs=2)
            nc.vector.tensor_scalar_mul(mu[:], bs_ps[:], 1.0 / DIM)
            musq = tmp.tile([P, 512], F32, name=f"musq{ln_idx}_{t}", tag="musq", bufs=2)
            nc.vector.tensor_mul(musq[:], mu[:], mu[:])
            # musq - EPS, so var = ex2 - musq + EPS below
            nc.vector.tensor_scalar_sub(musq[:], musq[:], EPS)
            var = tmp.tile([P, 512], F32, name=f"var{ln_idx}_{t}", tag="var", bufs=2)
            nc.vector.scalar_tensor_tensor(var[:], bq_ps[:], 1.0 / DIM, musq[:],
                                           mybir.AluOpType.mult, mybir.AluOpType.subtract)
            std = tmp.tile([P, 512], F32, name=f"std{ln_idx}_{t}", tag="std", bufs=2)
            nc.scalar.sqrt(std[:], var[:])
            rstd = tmp.tile([P, 512], F32, name=f"rstd{ln_idx}_{t}", tag="rstd", bufs=2)
            nc.vector.reciprocal_approx_fast(rstd[:], std[:])
            rstd16 = tmp.tile([P, 512], F16, name=f"rstd16{ln_idx}_{t}", tag="rstd16", bufs=2)
            nc.vector.tensor_copy(out=rstd16[:], in_=rstd[:])
            for c in range(KC):
                xm = tmp.tile([P, 512], F16, name=f"xm{ln_idx}_{t}_{c}", tag="xm", bufs=3)
                nc.vector.tensor_sub(xm[:], x_tiles[c][:, sl], mu[:])
                if trivial_aff:
                    nc.vector.tensor_mul(out_tiles[c][:, sl], xm[:], rstd16[:])
                else:
                    xn = tmp.tile([P, 512], F16, name=f"xn{ln_idx}_{t}_{c}", tag="xn", bufs=3)
                    nc.vector.tensor_mul(xn[:], xm[:], rstd16[:])
                    g_ap = aff[:, ln_idx * 20 + c: ln_idx * 20 + c + 1]
                    be_ap = aff[:, ln_idx * 20 + 10 + c: ln_idx * 20 + 10 + c + 1]
                    xg = tmp.tile([P, 512], F16, name=f"xg{ln_idx}_{t}_{c}", tag="xg", bufs=3)
                    nc.vector.tensor_scalar_mul(xg[:], xn[:], g_ap)
                    nc.scalar.activation(out_tiles[c][:, sl], xg[:], AF.Copy, bias=be_ap)

    def attn_finish(head, ops_, ov_psum, out_ap):
        den = tmp.tile([1, 512], F32, name=f"den{head}", tag="den", bufs=3)
        nc.scalar.copy(den[:], ops_[DHEAD:DHEAD + 1, :])
        rec32 = tmp.tile([1, 512], F32, name=f"rec32_{head}", tag="rec32", bufs=3)
        nc.vector.reciprocal_approx_fast(rec32[:], den[:])
        rec = tmp.tile([1, 512], F16, name=f"rec{head}", tag="rec", bufs=3)
        nc.vector.tensor_copy(out=rec[:], in_=rec32[:])
        bps = ov_psum.tile([DHEAD, 512], F32, name=f"bps{head}", tag="ov", bufs=4)
        nc.tensor.matmul(bps[:], ones_row[:, :DHEAD], rec[:],
                         start=True, stop=True)
        bsb = tmp.tile([DHEAD, 512], F32, name=f"bsb{head}", tag="bsb", bufs=3)
        nc.vector.tensor_copy(out=bsb[:], in_=bps[:])
        nc.vector.tensor_mul(out_ap, ops_[:DHEAD, :], bsb[:])

    def attn_pipeline(Kt_, Qt_, Vt_, n_kc, kv_par, sc_psum, ov_psum, epool, O_out):
        """Both heads of pair c share one two-bank score PSUM tile per key
        chunk (one exp instruction covers both heads), and the previous
        pair's attnV matmuls are interleaved with this pair's score matmuls
        at key-chunk granularity so the PE streams at the ACT exp pace."""
        prev = None  # (pair_idx, exps list of [kv_par, 1024] fp16 tiles)

        def attn_v_finish(pc, exps):
            ovs = []
            for h in range(2):
                head = 2 * pc + h
                ops_ = ov_psum.tile([DHEAD + 1, 512], F32, name=f"ov{head}", tag="ov",
                                    bufs=4)
                for k8 in range(n_kc):
                    nc.tensor.matmul(ops_[:],
                                     Vt_[k8][:kv_par, head, :],
                                     exps[k8][:, h * 512:(h + 1) * 512],
                                     start=(k8 == 0), stop=(k8 == n_kc - 1))
                ovs.append(ops_)
            attn_finish(2 * pc, ovs[0], ov_psum, O_out[pc][0:DHEAD, :])
            attn_finish(2 * pc + 1, ovs[1], ov_psum, O_out[pc][DHEAD:2 * DHEAD, :])

        for c in range(KC):
            exps = []
            for k8 in range(n_kc):
                sps = sc_psum.tile([kv_par, 1024], F32, name=f"sps{c}_{k8}", tag="sc",
                                   bufs=2)
                for h in range(2):
                    nc.tensor.matmul(sps[:, h * 512:(h + 1) * 512],
                                     Kt_[c][64 * h:64 * h + 64,
                                            k8 * kv_par:(k8 + 1) * kv_par],
                                     Qt_[c][64 * h:64 * h + 64, :],
                                     start=True, stop=True, tile_position=(64 * h, 0))
                e = epool.tile([kv_par, 1024], F16, name=f"exp{c}_{k8}", tag="exp")
                nc.scalar.activation(e[:], sps[:], AF.Exp, scale=SCALE)
                exps.append(e)
                if prev is not None and n_kc > 1:
                    # interleave previous pair's attnV with this pair's scores
                    pc, pexps = prev
                    for h in range(2):
                        nc.tensor.matmul(prev_ov[h][:],
                                         Vt_[k8][:kv_par, 2 * pc + h, :],
                                         pexps[k8][:, h * 512:(h + 1) * 512],
                                         start=(k8 == 0), stop=(k8 == n_kc - 1))
            if prev is not None:
                pc, pexps = prev
                if n_kc > 1:
                    attn_finish(2 * pc, prev_ov[0], ov_psum, O_out[pc][0:DHEAD, :])
                    attn_finish(2 * pc + 1, prev_ov[1], ov_psum,
                                O_out[pc][DHEAD:2 * DHEAD, :])
                else:
                    attn_v_finish(pc, pexps)
            if n_kc > 1:
                prev_ov = [ov_psum.tile([DHEAD + 1, 512], F32, name=f"ov{2*c+h}",
                                        tag="ov", bufs=4) for h in range(2)]
            prev = (c, exps)
        pc, pexps = prev
        if n_kc > 1:
            for k8 in range(n_kc):
                for h in range(2):
                    nc.tensor.matmul(prev_ov[h][:],
                                     Vt_[k8][:kv_par, 2 * pc + h, :],
                                     pexps[k8][:, h * 512:(h + 1) * 512],
                                     start=(k8 == 0), stop=(k8 == n_kc - 1))
            attn_finish(2 * pc, prev_ov[0], ov_psum, O_out[pc][0:DHEAD, :])
            attn_finish(2 * pc + 1, prev_ov[1], ov_psum, O_out[pc][DHEAD:2 * DHEAD, :])
        else:
            attn_v_finish(pc, pexps)

    def project(w_d, n_kc, rhs_fn, n_mc, consume, wpool, wtag, psum_p, wbufs=3):
        """out[mc] = sum_kc w[mc][:, kc].T @ rhs(kc); consume(mc, psum)."""
        for mc in range(n_mc):
            wt = wpool.tile([P, n_kc, P], F16, name=f"{wtag}_{mc}", tag=wtag, bufs=wbufs)
            nc.sync.dma_start(wt[:], w_d[mc])
            ps = psum_p.tile([P, 512], F32, name=f"ps_{wtag}_{mc}", tag="proj", bufs=4)
            for kc in range(n_kc):
                nc.tensor.matmul(ps[:], wt[:, kc], rhs_fn(kc),
                                 start=(kc == 0), stop=(kc == n_kc - 1))
            consume(mc, ps)

    def bias_ap(col):
        return biases[:, col:col + 1]

    # ---------------- phase 1: load x, LN1 ----------------

    resp = pool("resp", 1)
    ln1p = pool("ln1p", 1)
    ln1t = [ln1p.tile([P, TKV], F16, name=f"ln1_{c}", tag="ln1", bufs=KC) for c in range(KC)]

    ln_psum = pool("ln_psum", 1, space="PSUM")
    xpool = pool("xpool", 1, side="right")
    x_sb = []
    for c in range(KC):
        xc = xpool.tile([P, TKV], F32, name=f"x_{c}", tag="x", bufs=KC)
        nc.sync.dma_start(xc[:], d["xt"][c * P:(c + 1) * P, :])
        x_sb.append(xc)
    resid = []
    for c in range(KC):
        rc = resp.tile([P, T], F32, name=f"res_{c}", tag="res", bufs=KC)
        nc.scalar.copy(rc[:], x_sb[c][:, 0:T])
        resid.append(rc)
    layernorm(x_sb, TKV, 0, ln1t, ln_psum)
    close("xpool", "ln_psum")

    # ---------------- phase 2: Q, K, V projections ----------------

    proj_psum = pool("proj_psum", 1, space="PSUM")
    wpool = pool("wpool1", 1)
    qkv = pool("qkv", 1, side="right")

    Qt = [qkv.tile([P, T], F16, name=f"qt_{mc}", tag="qt", bufs=KC) for mc in range(KC)]
    Kt = [qkv.tile([P, TKV], F16, name=f"kt_{mc}", tag="kt", bufs=KC) for mc in range(KC)]
    Vt = [qkv.tile([P, HEADS, DHEAD + 1], F16, name=f"vt_{t8}", tag="vt", bufs=8)
          for t8 in range(8)]

    def q_consume(mc, ps):
        nc.any.tensor_copy(out=Qt[mc][:], in_=ps[:])

    project(d["wq1"], KC, lambda kc: ln1t[kc][:, 0:T], KC, q_consume, wpool, "wq1", proj_psum)

    for thalf in range(2):
        sl = slice(thalf * 512, (thalf + 1) * 512)

        def k_consume(mc, ps, sl=sl):
            nc.any.tensor_copy(out=Kt[mc][:, sl], in_=ps[:])

        project(d["wk1"], KC, lambda kc, sl=sl: ln1t[kc][:, sl], KC, k_consume, wpool, "wk1",
                proj_psum)

    # V token-major with a ones column per head (softmax denominator trick)
    wv_sb = []
    for kc in range(KC):
        wv = wpool.tile([P, DIM], F16, name=f"wv1_{kc}", tag="wv1", bufs=KC)
        nc.sync.dma_start(wv[:], d["wv1"][kc])
        wv_sb.append(wv)
    for t8 in range(8):
        nc.vector.memset(Vt[t8][:], 1.0)
        for n0, nsz in ((0, 512), (512, 512), (1024, 256)):
            ps = proj_psum.tile([P, 512], F32, name=f"psv_{t8}_{n0}", tag="proj", bufs=4)
            for kc in range(KC):
                nc.tensor.matmul(ps[:, :nsz], ln1t[kc][:, t8 * P:(t8 + 1) * P],
                                 wv_sb[kc][:, n0:n0 + nsz],
                                 start=(kc == 0), stop=(kc == KC - 1))
            nc.any.tensor_copy(
                out=Vt[t8][:, n0 // DHEAD:(n0 + nsz) // DHEAD, 0:DHEAD],
                in_=ps[:, :nsz].rearrange("p (h e) -> p h e", e=DHEAD))

    close("wpool1", "ln1p", "proj_psum")

    # ---------------- phase 3: attn1 ----------------

    otp = pool("otp", 1)
    Ot = [otp.tile([P, T], F16, name=f"ot_{c}", tag="ot", bufs=KC) for c in range(KC)]
    sc_psum = pool("sc_psum", 1, space="PSUM")
    ov_psum = pool("ov_psum", 1, space="PSUM")
    epool = pool("epool", 16, side="right")

    attn_pipeline(Kt, Qt, Vt, 8, P, sc_psum, ov_psum, epool, Ot)

    close("epool", "qkv", "ov_psum", "sc_psum")

    # ---------------- phase 4: out-proj 1 + residual ----------------

    wpool = pool("wpool2", 1)
    proj_psum = pool("proj_psum2", 1, space="PSUM")
    x1p = pool("x1p", 1, side="right")
    x1 = [x1p.tile([P, T], F32, name=f"x1_{mc}", tag="x1", bufs=KC) for mc in range(KC)]

    def o1_consume(mc, ps):
        if trivial_bias:
            nc.vector.tensor_add(x1[mc][:], ps[:], resid[mc][:])
        else:
            nc.vector.scalar_tensor_tensor(x1[mc][:], ps[:], bias_ap(mc), resid[mc][:],
                                           mybir.AluOpType.add, mybir.AluOpType.add)

    project(d["wo1"], KC, lambda kc: Ot[kc][:], KC, o1_consume, wpool, "wo1", proj_psum)
    close("wpool2", "otp", "resp", "proj_psum2")

    # ---------------- phase 5: LN2 + attn2 projections ----------------

    ln_psum = pool("ln_psum2", 1, space="PSUM")
    ln2p = pool("ln2p", 1)
    ln2t = [ln2p.tile([P, T], F16, name=f"ln2_{c}", tag="ln2", bufs=KC) for c in range(KC)]
    layernorm(x1, T, 1, ln2t, ln_psum)
    close("ln_psum2")

    proj_psum = pool("proj_psum2b", 1, space="PSUM")
    qkv2 = pool("qkv2", 1, side="right")
    wpool = pool("wpool2b", 1)
    ctx_sb = []
    for c in range(KCX):
        cc = qkv2.tile([P, MCTX], F32, name=f"ctx_{c}", tag="ctx", bufs=KCX)
        nc.sync.dma_start(cc[:], d["ctxt"][c * P:(c + 1) * P, :])
        ch = qkv2.tile([P, MCTX], F16, name=f"ctxh_{c}", tag="ctxh", bufs=KCX)
        nc.any.tensor_copy(out=ch[:], in_=cc[:])
        ctx_sb.append(ch)

    Q2t = [qkv2.tile([P, T], F16, name=f"q2t_{mc}", tag="q2t", bufs=KC) for mc in range(KC)]
    K2t = [qkv2.tile([P, MCTX], F16, name=f"k2t_{mc}", tag="k2t", bufs=KC) for mc in range(KC)]
    V2t = [qkv2.tile([P, HEADS, DHEAD + 1], F16, name="v2t", tag="v2t", bufs=1)]

    def q2_consume(mc, ps):
        nc.any.tensor_copy(out=Q2t[mc][:], in_=ps[:])

    project(d["wq2"], KC, lambda kc: ln2t[kc][:], KC, q2_consume, wpool, "wq2", proj_psum)

    for mc in range(KC):
        wt = wpool.tile([P, KCX, P], F16, name=f"wk2_{mc}", tag="wk2", bufs=3)
        nc.sync.dma_start(wt[:], d["wk2"][mc])
        ps = proj_psum.tile([P, MCTX], F32, name=f"psk2_{mc}", tag="projx", bufs=2)
        for kc in range(KCX):
            nc.tensor.matmul(ps[:], wt[:, kc], ctx_sb[kc][:], start=(kc == 0),
                             stop=(kc == KCX - 1))
        nc.any.tensor_copy(out=K2t[mc][:], in_=ps[:])

    wv2_sb = []
    for kc in range(KCX):
        wv = wpool.tile([P, DIM], F16, name=f"wv2_{kc}", tag="wv2", bufs=KCX)
        nc.sync.dma_start(wv[:], d["wv2"][kc])
        wv2_sb.append(wv)
    nc.vector.memset(V2t[0][:], 1.0)
    for n0, nsz in ((0, 512), (512, 512), (1024, 256)):
        ps = proj_psum.tile([MCTX, 512], F32, name=f"psv2_{n0}", tag="proj", bufs=4)
        for kc in range(KCX):
            nc.tensor.matmul(ps[:, :nsz], ctx_sb[kc][:], wv2_sb[kc][:, n0:n0 + nsz],
                             start=(kc == 0), stop=(kc == KCX - 1))
        nc.any.tensor_copy(
            out=V2t[0][:MCTX, n0 // DHEAD:(n0 + nsz) // DHEAD, 0:DHEAD],
            in_=ps[:, :nsz].rearrange("p (h e) -> p h e", e=DHEAD))

    close("wpool2b", "ln2p", "proj_psum2b")

    # ---------------- phase 6: attn2 ----------------

    o2p = pool("o2p", 1)
    O2t = [o2p.tile([P, T], F16, name=f"o2t_{c}", tag="o2t", bufs=KC) for c in range(KC)]
    sc_psum = pool("sc_psum2", 1, space="PSUM")
    ov_psum = pool("ov_psum2", 1, space="PSUM")
    epool = pool("epool2", 6, side="right")

    attn_pipeline(K2t, Q2t, V2t, 1, MCTX, sc_psum, ov_psum, epool, O2t)

    close("epool2", "qkv2", "ov_psum2", "sc_psum2")

    # ---------------- phase 7: out-proj 2 + residual ----------------

    x2p = pool("x2p", 1)
    wpool = pool("wpool3", 1)
    proj_psum = pool("proj_psum3", 1, space="PSUM")
    x2 = [x2p.tile([P, T], F32, name=f"x2_{mc}", tag="x2", bufs=KC) for mc in range(KC)]

    def o2_consume(mc, ps):
        if trivial_bias:
            nc.vector.tensor_add(x2[mc][:], ps[:], x1[mc][:])
        else:
            nc.vector.scalar_tensor_tensor(x2[mc][:], ps[:], bias_ap(10 + mc), x1[mc][:],
                                           mybir.AluOpType.add, mybir.AluOpType.add)

    project(d["wo2"], KC, lambda kc: O2t[kc][:], KC, o2_consume, wpool, "wo2", proj_psum)
    close("wpool3", "x1p", "proj_psum3")

    # ---------------- phase 8: LN3 + GEGLU FF ----------------

    hhp = pool("hhp", 1)
    hht = [hhp.tile([P, T], F16, name=f"hh_{j}", tag="hh", bufs=JFF) for j in range(JFF)]

    ln_psum = pool("ln_psum3", 1, space="PSUM")
    ln3p = pool("ln3p", 1)
    ln3t = [ln3p.tile([P, T], F16, name=f"ln3_{c}", tag="ln3", bufs=KC) for c in range(KC)]
    layernorm(x2, T, 2, ln3t, ln_psum)
    close("ln_psum3")

    wpool = pool("wpool4a", 1)
    proj_psum = pool("proj_psum4", 1, space="PSUM")
    for j in range(JFF):
        wg = wpool.tile([P, KC, P], F16, name=f"wg_{j}", tag="wff1g", bufs=3)
        nc.sync.dma_start(wg[:], d["wff1"][JFF + j])
        gps = proj_psum.tile([P, 512], F32, name=f"gps_{j}", tag="proj", bufs=4)
        for kc in range(KC):
            nc.tensor.matmul(gps[:], wg[:, kc], ln3t[kc][:], start=(kc == 0),
                             stop=(kc == KC - 1))
        gel = tmp.tile([P, T], F16, name=f"gel_{j}", tag="gel", bufs=3)
        if trivial_bias:
            nc.scalar.activation(gel[:], gps[:], AF.Gelu_apprx_tanh)
        else:
            nc.scalar.activation(gel[:], gps[:], AF.Gelu_apprx_tanh, bias=bias_ap(60 + j))

        wa = wpool.tile([P, KC, P], F16, name=f"wa_{j}", tag="wff1a", bufs=3)
        nc.sync.dma_start(wa[:], d["wff1"][j])
        aps = proj_psum.tile([P, 512], F32, name=f"aps_{j}", tag="proj", bufs=4)
        for kc in range(KC):
            nc.tensor.matmul(aps[:], wa[:, kc], ln3t[kc][:], start=(kc == 0),
                             stop=(kc == KC - 1))
        if trivial_bias:
            nc.vector.tensor_mul(hht[j][:], aps[:], gel[:])
        else:
            nc.vector.scalar_tensor_tensor(hht[j][:], aps[:], bias_ap(20 + j), gel[:],
                                           mybir.AluOpType.add, mybir.AluOpType.mult)

    close("wpool4a", "ln3p")

    # ---------------- phase 9: FF down-proj + residual -> out ----------------

    wpool = pool("wpool4b", 1)
    outp = pool("outp", 4)
    for mc in range(KC):
        wt = wpool.tile([P, JFF, P], F16, name=f"wff2_{mc}", tag="wff2", bufs=2)
        nc.sync.dma_start(wt[:], d["wff2"][mc])
        ps = proj_psum.tile([P, 512], F32, name=f"psf2_{mc}", tag="proj", bufs=4)
        for kc in range(JFF):
            nc.tensor.matmul(ps[:], wt[:, kc], hht[kc][:], start=(kc == 0),
                             stop=(kc == JFF - 1))
        ot = outp.tile([P, T], F32, name=f"out_{mc}", tag="out")
        if trivial_bias:
            nc.vector.tensor_add(ot[:], ps[:], x2[mc][:])
        else:
            nc.vector.scalar_tensor_tensor(ot[:], ps[:], bias_ap(100 + mc), x2[mc][:],
                                           mybir.AluOpType.add, mybir.AluOpType.add)
        nc.sync.dma_start(d["out"][mc * P:(mc + 1) * P, :], ot[:])

    close("outp", "wpool4b", "hhp", "x2p", "o2p", "tmp", "const", "proj_psum4")


def _lhst_layout(w, n_kc, n_mc):
    """[K, M] f32 -> fp16 [n_mc, 128, n_kc, 128] so block [mc] is the
    contiguous stationary-operand group for output chunk mc."""
    return np.ascontiguousarray(
        w.reshape(n_kc, P, n_mc, P).transpose(2, 1, 0, 3).astype(np.float16))


def _rhs_layout(w, n_kc):
    """[K, M] f32 -> fp16 [n_kc, 128, M] row-chunk (moving-operand) layout."""
    return np.ascontiguousarray(w.reshape(n_kc, P, -1).astype(np.float16))


_BUILT = {}


def _build(trivial_aff, trivial_bias):
    key = (trivial_aff, trivial_bias)
    if key in _BUILT:
        return _BUILT[key]
    nc = bacc.Bacc("TRN2", target_bir_lowering=False, debug=False, num_devices=N_CORES)
    d = {
        "xt": nc.dram_tensor("xt", [DIM, TKV], F32, kind="ExternalInput").ap(),
        "ctxt": nc.dram_tensor("ctxt", [CTX_DIM, MCTX], F32, kind="ExternalInput").ap(),
        "wq1": nc.dram_tensor("wq1", [KC, P, KC, P], F16, kind="ExternalInput").ap(),
        "wk1": nc.dram_tensor("wk1", [KC, P, KC, P], F16, kind="ExternalInput").ap(),
        "wv1": nc.dram_tensor("wv1", [KC, P, DIM], F16, kind="ExternalInput").ap(),
        "wo1": nc.dram_tensor("wo1", [KC, P, KC, P], F16, kind="ExternalInput").ap(),
        "wq2": nc.dram_tensor("wq2", [KC, P, KC, P], F16, kind="ExternalInput").ap(),
        "wk2": nc.dram_tensor("wk2", [KC, P, KCX, P], F16, kind="ExternalInput").ap(),
        "wv2": nc.dram_tensor("wv2", [KCX, P, DIM], F16, kind="ExternalInput").ap(),
        "wo2": nc.dram_tensor("wo2", [KC, P, KC, P], F16, kind="ExternalInput").ap(),
        "wff1": nc.dram_tensor("wff1", [2 * JFF, P, KC, P], F16, kind="ExternalInput").ap(),
        "wff2": nc.dram_tensor("wff2", [KC, P, JFF, P], F16, kind="ExternalInput").ap(),
        "out": nc.dram_tensor("out", [DIM, T], F32, kind="ExternalOutput").ap(),
    }
    if not trivial_aff:
        d["aff"] = nc.dram_tensor("aff", [P, 60], F32, kind="ExternalInput").ap()
    if not trivial_bias:
        d["biases"] = nc.dram_tensor("biases", [P, 110], F32, kind="ExternalInput").ap()
    with tile.TileContext(nc) as tc:
        _emit(tc, d, trivial_aff, trivial_bias)
    nc.compile()
    _BUILT[key] = nc
    return nc


def kernel(x, context,
           g1, be1, wq1, wk1, wv1, wo1, bo1,
           g2, be2, wq2, wk2, wv2, wo2, bo2,
           g3, be3, w_ff1, b_ff1, w_ff2, b_ff2,
           _trace=False):
    global last_exec_time_ns
    x = np.asarray(x, np.float32)
    context = np.asarray(context, np.float32)

    affs = [np.asarray(a, np.float32) for a in (g1, be1, g2, be2, g3, be3)]
    biases = [np.asarray(b, np.float32) for b in (bo1, bo2, b_ff1, b_ff2)]
    trivial_aff = all(np.all(a == (1.0 if i % 2 == 0 else 0.0))
                      for i, a in enumerate(affs))
    trivial_bias = all(np.all(b == 0.0) for b in biases)

    nc = _build(trivial_aff, trivial_bias)

    shared = {
        "wq1": _lhst_layout(np.asarray(wq1, np.float32), KC, KC),
        "wk1": _lhst_layout(np.asarray(wk1, np.float32), KC, KC),
        "wv1": _rhs_layout(np.asarray(wv1, np.float32), KC),
        "wo1": _lhst_layout(np.asarray(wo1, np.float32), KC, KC),
        "wq2": _lhst_layout(np.asarray(wq2, np.float32), KC, KC),
        "wk2": _lhst_layout(np.asarray(wk2, np.float32), KCX, KC),
        "wv2": _rhs_layout(np.asarray(wv2, np.float32), KCX),
        "wo2": _lhst_layout(np.asarray(wo2, np.float32), KC, KC),
        "wff1": _lhst_layout(np.asarray(w_ff1, np.float32), KC, 2 * JFF),
        "wff2": _lhst_layout(np.asarray(w_ff2, np.float32), JFF, KC),
    }
    if not trivial_aff:
        aff = np.zeros([P, 60], np.float32)
        for i, a in enumerate(affs):
            # col = ln_idx*20 + (0 for g / 10 for be) + chunk
            ln_idx, j = i // 2, i % 2
            aff[:, ln_idx * 20 + j * 10: ln_idx * 20 + j * 10 + 10] = \
                a.reshape(KC, P).T
        shared["aff"] = aff
    if not trivial_bias:
        bb = np.zeros([P, 110], np.float32)
        bb[:, 0:10] = biases[0].reshape(KC, P).T
        bb[:, 10:20] = biases[1].reshape(KC, P).T
        bb[:, 20:100] = biases[2].reshape(2 * JFF, P).T
        bb[:, 100:110] = biases[3].reshape(KC, P).T
        shared["biases"] = bb

    in_maps = []
    for b in range(BATCH):
        ctxt = np.ascontiguousarray(context[b].T)
        for h in range(2):
            xr = np.roll(x[b], -h * T, axis=0)
            m = dict(shared)
            m["xt"] = np.ascontiguousarray(xr.T)
            m["ctxt"] = ctxt
            in_maps.append(m)

    res = bass_utils.run_bass_kernel_spmd(
        nc, in_maps, core_ids=list(range(N_CORES)), trace=_trace)
    last_exec_time_ns = res.exec_time_ns

    out = np.empty((BATCH, NTOK, DIM), np.float32)
    for b in range(BATCH):
        for h in range(2):
            out[b, h * T:(h + 1) * T, :] = res.results[b * 2 + h]["out"].T
    return out
